# revision 14
# baseline (speedup 1.0000x reference)
"""Trainium2 Bass kernel for nn_DAGModel (gnn_message_passing).

Strategy (data-parallel over batch, 8 b's per core):
- node_vecs live in DRAM as a bf16 table `nv[token, b8, h128]` (2KB rows,
  all 8 local batch elements interleaved per token so one gather serves
  all of them).
- Parent gathers use the GPSIMD bulk `dma_gather(transpose=True)` which
  lands feature-major ([h on partitions, gather-position on free]) —
  exactly the matmul layout.
- Each depth's tokens are reordered (host-side renumbering) by
  (#parents-from-older-depths desc, #parents desc). Parents from depths
  <= d-2 ("old") then form per-slot prefix lists, and parents from depth
  d-1 ("new") form per-(slot,block) prefix segments. Old and new refs are
  packed into separate gather bundles: old bundles depend only on
  write-backs through depth d-2, so the gather engine fetches them
  DURING depth d-1's compute; only the small new bundles wait for depth
  d-1's write-back. This removes most of the per-depth serialization.
- Bundles are packed across chunks per depth (fewer gather instructions,
  each <= BCAP indices; the HW path crashes above ~900).
- Depth 0 has only the root as a possible parent: pv = count * root,
  computed on-chip with no gathers.
- Node embeddings are statically known (node_indices is an input), so
  permuted embedding tiles are prepared host-side feature-major and
  streamed in — no embedding gathers.
- The 2-layer MLP runs in bf16 on the PE (h on partitions); the residual
  `pv` is injected into the second matmul's PSUM via an identity matmul.
- New vecs are PE-transposed (bf16, single pass) back to row-major and
  DMA'd to the next depth's token rows; the output projection
  out[t] = nv[t]·Wout[t] is a DVE multiply + reduce over the row-major
  tile, so the final [B,N] readout costs no extra pass over node_vecs.
"""

import numpy as np
import ml_dtypes

BF16 = ml_dtypes.bfloat16

# Full-problem dims (hardcoded per contract).
B, H, E = 64, 128, 128
D_FULL, P_FULL, MP = 20, 1000, 8
NCORES, BL = 8, 8
LAST_RESULTS = None


# ---------------------------------------------------------------------------
# workaround: this walrus build rejects >1 sync-wait on a CTRL (Drain) inst.
def _install_tilefix():
    import concourse.tile as tile_mod
    from concourse.vector_clock import ScopedClock, VectorClock

    if getattr(tile_mod.TileContext, "_drain_split_installed", False):
        return

    def _split_drain_and_barrier(self, tick_clock, wait_clock):
        gc = tick_clock.global_clock
        ticks = list(gc)
        nz = [(i, t) for i, t in enumerate(ticks) if t > 0]
        if nz:
            for i, t in nz:
                vec = [0] * len(ticks)
                vec[i] = t
                d = self.nc.sync.drain()
                wait_clock.add_sem_waits(
                    d.ins, ScopedClock({None: VectorClock(vec)})
                )
        else:
            d = self.nc.sync.drain()
            wait_clock.add_sem_waits(d.ins, ScopedClock({None: gc}))
        self.nc.all_engine_barrier()
        assert self.sems is not None
        popped = self.nc._tile_sem_poison_stack.pop()
        assert popped is self._sem_poison
        self.nc.clear_and_free_semaphores(list(self.sems.allocated().values()))
        self.nc.all_engine_barrier()

    tile_mod.TileContext._drain_and_barrier = _split_drain_and_barrier
    tile_mod.TileContext._drain_split_installed = True


# ---------------------------------------------------------------------------
def _wrap_idx(seq):
    """int16 index layout for dma_gather: position i -> [i%16, i//16],
    replicated across the 8 groups of 16 partitions."""
    a = np.asarray(seq, np.int16)
    L = len(a)
    assert L % 16 == 0
    a16 = a.reshape(L // 16, 16).T  # [16, L/16]
    return np.ascontiguousarray(np.tile(a16, (8, 1)))  # [128, L/16]


def _pack_bundles(items, bcap):
    """Pack (key, idx_list) items into bundles of <= bcap indices each.
    Returns (bundles, where): bundles = [concatenated index array ...];
    where[key] = (bundle_idx, offset, length)."""
    bundles = []
    where = {}
    cur, cur_len = [], 0
    for key, lst in items:
        n = len(lst)
        assert n <= bcap
        if cur_len + n > bcap and cur_len > 0:
            bundles.append(np.concatenate(cur))
            cur, cur_len = [], 0
        where[key] = (len(bundles), cur_len, n)
        cur.append(lst)
        cur_len += n
    if cur_len:
        bundles.append(np.concatenate(cur))
    return bundles, where


def _prepare(inputs, D, P, CH, NCH):
    """Host-side index preprocessing. Only index tensors and statically
    known embedding/weight reorders are transformed on the host; all
    batch-dependent float compute stays on device."""
    PP = CH * NCH
    node_indices = np.asarray(inputs["node_indices"])
    parent_indices = np.asarray(inputs["parent_indices"])
    k = (parent_indices > 0).sum(-1)  # [D, P]

    BCAP = 768
    remap = np.zeros(2 + D * P, np.int64)
    remap[1] = 1
    perms = []
    k_olds = []
    for d in range(D):
        if d == 0:
            perm = np.argsort(-k[0], kind="stable")
            k_old = np.zeros(P, np.int64)
        else:
            base_prev_orig = 2 + (d - 1) * P  # first original index of depth d-1
            k_old = ((parent_indices[d] > 0) & (parent_indices[d] < base_prev_orig)).sum(-1)
            perm = np.lexsort((-k[d], -k_old))
        perms.append(perm)
        k_olds.append(k_old)
        remap[2 + d * P + perm] = 2 + d * PP + np.arange(P)

    # Per depth: old bundles (parents in depths <= d-2; per-chunk per-slot
    # prefix lists) and new bundles (parents in depth d-1; per-(slot,block)
    # prefix segments). meta[d] = dict(bundles=[(L, maxref, is_old)...],
    # chunks=[ops...]) where ops are DVE add/copy instructions over pv.
    meta = []
    pidx_wrapped = {}  # (d, bi) -> wrapped idx
    MAXB = 0
    LBMAX = 128
    for d in range(D):
        if d == 0:
            meta.append({"bundles": [], "chunks": [None] * NCH})
            continue
        perm = perms[d]
        kd = k[d][perm]
        kod = k_olds[d][perm]
        knd = kd - kod
        old_items = []  # ((c, 'o', j), list)
        new_items = []  # ((c, 'n', sid), list)
        chunk_ops = []  # per chunk: list of op dicts
        for c in range(NCH):
            lo = c * CH
            hi = min(lo + CH, P)
            nodes = perm[lo:hi]
            ko = kod[lo:hi]
            kn = knd[lo:hi]
            ops = []
            # --- old: slot-major prefix lists
            m_old = [int((ko > j).sum()) for j in range(MP)]
            for j in range(MP):
                if m_old[j] == 0:
                    break
                lst = remap[parent_indices[d, nodes[: m_old[j]], j]]
                old_items.append(((c, "o", j), lst))
            m0, m1 = m_old[0], m_old[1]
            if m1 > 0:
                ops.append({"op": "add2", "lo": 0, "n": m1,
                            "a": (c, "o", 0), "b": (c, "o", 1)})
            if m0 > m1:
                ops.append({"op": "copy", "lo": m1, "n": m0 - m1,
                            "a": (c, "o", 0), "ao": m1})
            if m0 < CH:
                ops.append({"op": "zero", "lo": m0, "n": CH - m0})
            for j in range(2, MP):
                if m_old[j] == 0:
                    break
                ops.append({"op": "add", "lo": 0, "n": m_old[j], "a": (c, "o", j)})
            # --- new: per (new-slot jj, k_old block) prefix segments
            # block boundaries: runs of equal ko (desc); within runs kn desc.
            sid = 0
            bounds = [0] + list(np.nonzero(np.diff(ko))[0] + 1) + [hi - lo]
            for jj in range(MP):
                any_seg = False
                for bs, be in zip(bounds[:-1], bounds[1:]):
                    cnt = int((kn[bs:be] > jj).sum())
                    if cnt == 0:
                        continue
                    any_seg = True
                    K = int(ko[bs])
                    lst = remap[parent_indices[d, nodes[bs : bs + cnt], K + jj]]
                    new_items.append(((c, "n", sid), lst))
                    ops.append({"op": "add", "lo": bs, "n": cnt, "a": (c, "n", sid)})
                    sid += 1
                if not any_seg:
                    break
            chunk_ops.append(ops)

        old_bundles, old_where = _pack_bundles(old_items, BCAP)
        new_bundles, new_where = _pack_bundles(new_items, BCAP)
        nold = len(old_bundles)
        where = {kk: (bi, off, n) for kk, (bi, off, n) in old_where.items()}
        where.update(
            {kk: (nold + bi, off, n) for kk, (bi, off, n) in new_where.items()}
        )
        bundles = []
        for bi, idx in enumerate(old_bundles + new_bundles):
            L = (len(idx) + 127) // 128 * 128
            maxref = int(idx.max())
            idx = np.pad(idx, (0, L - len(idx)))
            pidx_wrapped[(d, bi)] = _wrap_idx(idx)
            bundles.append((L, maxref, bi < nold))
            LBMAX = max(LBMAX, L)
        MAXB = max(MAXB, len(bundles))
        meta.append({"bundles": bundles, "chunks": chunk_ops, "where": where,
                     "nold": nold})

    pidx_np = np.zeros((D, MAXB, 128, LBMAX // 16), np.int16)
    for (d, bi), w in pidx_wrapped.items():
        pidx_np[d, bi, :, : w.shape[1]] = w

    W1 = np.asarray(inputs["W1"], np.float32)
    W2 = np.asarray(inputs["W2"], np.float32)
    Wout = np.asarray(inputs["Wout"], np.float32)
    emb = np.asarray(inputs["emb_table"], np.float32)

    # permuted node-embedding tiles, feature-major [D, NCH, E, CH] (bf16)
    neT = np.zeros((D, NCH, H, CH), BF16)
    for d in range(D):
        for c in range(NCH):
            lo = c * CH
            hi = min(lo + CH, P)
            nodes = perms[d][lo:hi]
            ne = emb[node_indices[d][nodes]]  # [nreal, E] f32
            neT[d, c, :, : hi - lo] = ne.T.astype(BF16)

    wout_perm = np.zeros((D, PP, H), BF16)
    wo = Wout[1:].reshape(D, P, H)
    for d in range(D):
        wout_perm[d, :P] = wo[d][perms[d]].astype(BF16)

    # depth-0 parent counts in permuted order, replicated across partitions
    kcnt = np.zeros((NCH, 128, CH), BF16)
    k0 = np.pad(k[0][perms[0]].astype(np.float32), (0, PP - P))
    for c in range(NCH):
        kcnt[c] = np.tile(k0[c * CH : (c + 1) * CH].astype(BF16), (128, 1))

    prep = {
        "meta": meta,
        "perms": perms,
        "pidx": pidx_np,
        "neT": neT,
        "w1at": np.ascontiguousarray(W1[:, :H].T.astype(BF16)),
        "w1bt": np.ascontiguousarray(W1[:, H:].T.astype(BF16)),
        "w2t": np.ascontiguousarray(W2.T.astype(BF16)),
        "ident": np.eye(128, dtype=BF16),
        "b1": np.asarray(inputs["b1"], np.float32).reshape(128, 1),
        "b2": np.asarray(inputs["b2"], np.float32).reshape(128, 1),
        "woutp": wout_perm,
        "kcnt": kcnt,
    }
    return prep


def _build(prep, D, P, CH, NCH):
    """Trace the Bass/Tile kernel. Returns a finalized Bacc."""
    _install_tilefix()
    from contextlib import ExitStack

    import concourse.bacc as bacc
    import concourse.mybir as mybir
    from concourse.tile import TileContext

    PP = CH * NCH
    TOK = 2 + D * PP
    ROW = BL * H  # nv row elems (bf16)
    KB = CH // 128  # 128-blocks per chunk
    f32 = mybir.dt.float32
    bf16 = mybir.dt.bfloat16
    i16 = mybir.dt.int16
    AF = mybir.ActivationFunctionType
    ALU = mybir.AluOpType
    AX = mybir.AxisListType

    nc = bacc.Bacc("TRN2", target_bir_lowering=False, debug=False)

    nv = nc.dram_tensor("nv", [TOK, ROW], bf16, kind="Internal")
    nvinit = nc.dram_tensor("nvinit", [2, ROW], bf16, kind="ExternalInput")
    rootT_in = nc.dram_tensor("rootT", [128, BL], bf16, kind="ExternalInput")
    neT_in = nc.dram_tensor(
        "neT", list(prep["neT"].shape), bf16, kind="ExternalInput"
    )
    pidx_in = nc.dram_tensor(
        "pidx", list(prep["pidx"].shape), i16, kind="ExternalInput"
    )
    w1at_in = nc.dram_tensor("w1at", [128, 128], bf16, kind="ExternalInput")
    w1bt_in = nc.dram_tensor("w1bt", [128, 128], bf16, kind="ExternalInput")
    w2t_in = nc.dram_tensor("w2t", [128, 128], bf16, kind="ExternalInput")
    ident_in = nc.dram_tensor("ident", [128, 128], bf16, kind="ExternalInput")
    b1_in = nc.dram_tensor("b1c", [128, 1], f32, kind="ExternalInput")
    b2_in = nc.dram_tensor("b2c", [128, 1], f32, kind="ExternalInput")
    woutp_in = nc.dram_tensor("woutp", [D, PP, H], bf16, kind="ExternalInput")
    kcnt_in = nc.dram_tensor("kcnt", [NCH, 128, CH], bf16, kind="ExternalInput")
    outd = nc.dram_tensor("outd", [D, NCH, 128, KB, BL], bf16, kind="ExternalOutput")

    meta = prep["meta"]

    with TileContext(nc) as tc, ExitStack() as ctx:
        const = ctx.enter_context(tc.tile_pool(name="const", bufs=1))
        pidx_pool = ctx.enter_context(tc.tile_pool(name="pidx", bufs=2))
        stag_pool = ctx.enter_context(tc.tile_pool(name="stag", bufs=1))
        pv_pool = ctx.enter_context(tc.tile_pool(name="pv", bufs=2))
        ne_pool = ctx.enter_context(tc.tile_pool(name="ne", bufs=2))
        h1_pool = ctx.enter_context(tc.tile_pool(name="h1", bufs=3))
        nvn_pool = ctx.enter_context(tc.tile_pool(name="nvn", bufs=6))
        nvrm_pool = ctx.enter_context(tc.tile_pool(name="nvrm", bufs=3))
        wout_pool = ctx.enter_context(tc.tile_pool(name="wout", bufs=3))
        outsb_pool = ctx.enter_context(tc.tile_pool(name="outsb", bufs=2))
        tmp_pool = ctx.enter_context(tc.tile_pool(name="tmp", bufs=2))
        psum_mm = ctx.enter_context(tc.tile_pool(name="psmm", bufs=2, space="PSUM"))
        psum_mm2 = ctx.enter_context(tc.tile_pool(name="psm2", bufs=2, space="PSUM"))
        psum_tp = ctx.enter_context(tc.tile_pool(name="pstp", bufs=4, space="PSUM"))

        w1at = const.tile([128, 128], bf16)
        nc.sync.dma_start(out=w1at[:], in_=w1at_in[:, :])
        w1bt = const.tile([128, 128], bf16)
        nc.sync.dma_start(out=w1bt[:], in_=w1bt_in[:, :])
        w2t = const.tile([128, 128], bf16)
        nc.sync.dma_start(out=w2t[:], in_=w2t_in[:, :])
        ident = const.tile([128, 128], bf16)
        nc.sync.dma_start(out=ident[:], in_=ident_in[:, :])
        b1 = const.tile([128, 1], f32)
        nc.sync.dma_start(out=b1[:], in_=b1_in[:, :])
        b2 = const.tile([128, 1], f32)
        nc.sync.dma_start(out=b2[:], in_=b2_in[:, :])
        rootT = const.tile([128, BL], bf16)
        nc.sync.dma_start(out=rootT[:], in_=rootT_in[:, :])

        # init nv rows 0..1 (zero pad row + root = per-b embedding)
        nvi = const.tile([2, ROW], bf16)
        nc.sync.dma_start(out=nvi[:], in_=nvinit[:, :])
        nc.sync.dma_start(out=nv[0:2, :], in_=nvi[:])

        stags = {}  # d -> list of stag tiles
        for d in range(D):
            md = meta[d]
            # ---- gathers for this depth (old bundles first: they only
            # depend on write-backs through depth d-2, so the gather engine
            # runs them during depth d-1's compute).
            dst = []
            for bi, (L, maxref, is_old) in enumerate(md["bundles"]):
                pidx_sb = pidx_pool.tile([128, L // 16], i16, tag=f"pidx{bi}")
                nc.sync.dma_start(out=pidx_sb[:], in_=pidx_in[d, bi, :, : L // 16])
                stag = stag_pool.tile([128, BL, L], bf16, tag=f"gb{bi}")
                nc.gpsimd.dma_gather(
                    stag[:], nv[0 : maxref + 1, :], pidx_sb[:],
                    num_idxs=L, num_idxs_reg=L,
                    elem_size=ROW, transpose=True,
                )
                dst.append(stag)
            stags[d] = dst

            for c in range(NCH):
                # ---- parent-sum -> pv bf16 [128, BL, CH]
                pv = pv_pool.tile([128, BL, CH], bf16)
                if d == 0:
                    kc = ne_pool.tile([128, 1, CH], bf16, tag="kcnt")
                    nc.sync.dma_start(out=kc[:], in_=kcnt_in[c, :, :].unsqueeze(1))
                    nc.vector.tensor_tensor(
                        out=pv[:],
                        in0=rootT[:].unsqueeze(2).to_broadcast([128, BL, CH]),
                        in1=kc[:].to_broadcast([128, BL, CH]),
                        op=ALU.mult,
                    )
                else:
                    where = md["where"]

                    def seg(key, off0, n):
                        bi, off, ln = where[key]
                        assert off0 + n <= ln
                        return stags[d][bi][:, :, off + off0 : off + off0 + n]

                    for op in md["chunks"][c]:
                        lo, n = op["lo"], op["n"]
                        dstv = pv[:, :, lo : lo + n]
                        if op["op"] == "add2":
                            nc.vector.tensor_add(
                                dstv, seg(op["a"], 0, n), seg(op["b"], 0, n)
                            )
                        elif op["op"] == "copy":
                            nc.vector.tensor_copy(
                                out=dstv, in_=seg(op["a"], op["ao"], n)
                            )
                        elif op["op"] == "zero":
                            nc.vector.memset(dstv, 0)
                        elif op["op"] == "add":
                            nc.vector.tensor_add(dstv, dstv, seg(op["a"], 0, n))

                ne = ne_pool.tile([128, 1, CH], bf16)
                nc.sync.dma_start(out=ne[:], in_=neT_in[d, c, :, :].unsqueeze(1))

                # ---- MLP (bf16) over col pairs (2 b's x CH = 512 cols)
                nvns = []
                for bp in range(BL // 2):
                    rhs_pv = pv[:, 2 * bp : 2 * bp + 2, :]
                    h1p = psum_mm.tile([128, 2, CH], f32, tag="h1p")
                    nc.tensor.matmul(
                        h1p[:], lhsT=w1at[:], rhs=rhs_pv, start=True, stop=False
                    )
                    nc.tensor.matmul(
                        h1p[:],
                        lhsT=w1bt[:],
                        rhs=ne[:].to_broadcast([128, 2, CH]),
                        start=False,
                        stop=True,
                    )
                    h1 = h1_pool.tile([128, 2, CH], bf16)
                    nc.scalar.activation(h1[:], h1p[:], AF.Relu, bias=b1[:])
                    h2p = psum_mm2.tile([128, 2, CH], f32, tag="h2p")
                    nc.tensor.matmul(
                        h2p[:], lhsT=w2t[:], rhs=h1[:], start=True, stop=False
                    )
                    nc.tensor.matmul(
                        h2p[:], lhsT=ident[:], rhs=rhs_pv, start=False, stop=True
                    )
                    nvn = nvn_pool.tile([128, 2, CH], bf16)
                    if bp < 2:
                        nc.scalar.activation(nvn[:], h2p[:], AF.Identity, bias=b2[:])
                    else:
                        nc.vector.tensor_scalar(
                            out=nvn[:], in0=h2p[:], scalar1=b2[:], scalar2=None,
                            op0=ALU.add,
                        )
                    nvns.append(nvn)

                # ---- transpose back (bf16), write-back, fused out-projection
                outsb = outsb_pool.tile([128, KB * BL], bf16)
                for kb in range(KB):
                    kbg = c * KB + kb
                    wout_sb = wout_pool.tile([128, 128], bf16)
                    nc.sync.dma_start(
                        out=wout_sb[:],
                        in_=woutp_in[d, kbg * 128 : (kbg + 1) * 128, :],
                    )
                    nvrm = nvrm_pool.tile([128, BL, 128], bf16)
                    for half in range(2):
                        tp = psum_tp.tile([128, 4, 128], bf16, tag="tp")
                        for bq in range(4):
                            b = half * 4 + bq
                            nc.tensor.transpose(
                                tp[:, bq, :],
                                nvns[b // 2][:, b % 2, kb * 128 : (kb + 1) * 128],
                                ident[:],
                            )
                        nc.scalar.copy(
                            out=nvrm[:, half * 4 : half * 4 + 4, :], in_=tp[:]
                        )
                    # out-projection: out[t, b] = sum_h nvrm[t,b,h]*wout[t,h]
                    tmp = tmp_pool.tile([128, BL, 128], bf16)
                    nc.vector.tensor_tensor(
                        out=tmp[:],
                        in0=nvrm[:],
                        in1=wout_sb[:].unsqueeze(1).to_broadcast([128, BL, 128]),
                        op=ALU.mult,
                    )
                    with nc.allow_low_precision("bf16 out-projection partial"):
                        nc.vector.tensor_reduce(
                            out=outsb[:, kb * BL : (kb + 1) * BL],
                            in_=tmp[:],
                            axis=AX.X,
                            op=ALU.add,
                        )
                    tokbase = 2 + d * PP + c * CH + kb * 128
                    nc.sync.dma_start(
                        out=nv[tokbase : tokbase + 128, :],
                        in_=nvrm[:].rearrange("p b h -> p (b h)"),
                    )
                nc.sync.dma_start(
                    out=outd[d, c, :, :, :],
                    in_=outsb[:].rearrange("p (k b) -> p k b", k=KB),
                )

    nc.finalize()
    return nc


def _run_cores(nc, prep, embedding, n_cores):
    from concourse import bass_utils

    in_maps = []
    base = {
        "neT": prep["neT"],
        "pidx": prep["pidx"],
        "w1at": prep["w1at"],
        "w1bt": prep["w1bt"],
        "w2t": prep["w2t"],
        "ident": prep["ident"],
        "b1c": prep["b1"],
        "b2c": prep["b2"],
        "woutp": prep["woutp"],
        "kcnt": prep["kcnt"],
    }
    for core in range(n_cores):
        eb = embedding[core * BL : (core + 1) * BL]  # [BL, H]
        nvinit = np.zeros((2, BL * H), np.float32)
        nvinit[1] = eb.reshape(-1)
        m = dict(base)
        m["nvinit"] = np.ascontiguousarray(nvinit.astype(BF16))
        m["rootT"] = np.ascontiguousarray(eb.T.astype(BF16))
        in_maps.append(m)
    res = bass_utils.run_bass_kernel_spmd(
        nc, in_maps, core_ids=list(range(n_cores))
    )
    global LAST_RESULTS
    LAST_RESULTS = res
    return res


def _assemble(results, prep, inputs, D, P, CH, NCH, n_cores):
    PP = CH * NCH
    KB = CH // 128
    embedding = np.asarray(inputs["embedding"], np.float32)
    Wout = np.asarray(inputs["Wout"], np.float32)
    bout = np.asarray(inputs["bout"], np.float32)
    NTOT = 1 + D * P

    out = np.empty((embedding.shape[0], NTOT), np.float32)
    out[:, 0] = embedding @ Wout[0] + bout[0]
    for core in range(n_cores):
        v = np.asarray(results[core]["outd"], np.float32)  # [D, NCH, 128, KB, BL]
        v = v.transpose(0, 1, 3, 2, 4).reshape(D, PP, BL)  # s = c*CH + kb*128 + n
        for d in range(D):
            perm = prep["perms"][d]
            cols = 1 + d * P + perm  # output column for sorted position s
            out[core * BL : (core + 1) * BL, cols] = v[d, :P].T
    out[:, 1:] += bout[None, 1:]
    return out


def kernel(**inputs):
    D, P, CH, NCH = D_FULL, P_FULL, 256, 4
    prep = _prepare(inputs, D, P, CH, NCH)
    nc = _build(prep, D, P, CH, NCH)
    res = _run_cores(nc, prep, np.asarray(inputs["embedding"], np.float32), NCORES)
    return _assemble(res.results, prep, inputs, D, P, CH, NCH, NCORES)


# revision 19
# speedup vs baseline: 1.0812x; 1.0812x over previous
"""Trainium2 Bass kernel for nn_DAGModel (gnn_message_passing).

Strategy (data-parallel over batch, 8 b's per core):
- node_vecs live in DRAM as a bf16 table `nv[token, b8, h128]` (2KB rows,
  all 8 local batch elements interleaved per token so one gather serves
  all of them).
- Parent gathers use the GPSIMD bulk `dma_gather(transpose=True)` which
  lands feature-major ([h on partitions, gather-position on free]) —
  exactly the matmul layout.
- Each depth's tokens are reordered (host-side renumbering) by
  (#parents-from-older-depths desc, #parents desc). Parents from depths
  <= d-2 ("old") then form per-slot prefix lists, and parents from depth
  d-1 ("new") form per-(slot,block) prefix segments. Old and new refs are
  packed into separate gather bundles: old bundles depend only on
  write-backs through depth d-2, so the gather engine fetches them
  DURING depth d-1's compute; only the small new bundles wait for depth
  d-1's write-back. This removes most of the per-depth serialization.
- Bundles are packed across chunks per depth (fewer gather instructions,
  each <= BCAP indices; the HW path crashes above ~900).
- Depth 0 has only the root as a possible parent: pv = count * root,
  computed on-chip with no gathers.
- Node embeddings are statically known (node_indices is an input), so
  permuted embedding tiles are prepared host-side feature-major and
  streamed in — no embedding gathers.
- The 2-layer MLP runs in bf16 on the PE (h on partitions); the residual
  `pv` is injected into the second matmul's PSUM via an identity matmul.
- New vecs are PE-transposed (bf16, single pass) back to row-major and
  DMA'd to the next depth's token rows; the output projection
  out[t] = nv[t]·Wout[t] is a DVE multiply + reduce over the row-major
  tile, so the final [B,N] readout costs no extra pass over node_vecs.
"""

import numpy as np
import ml_dtypes

BF16 = ml_dtypes.bfloat16

# Full-problem dims (hardcoded per contract).
B, H, E = 64, 128, 128
D_FULL, P_FULL, MP = 20, 1000, 8
NCORES, BL = 8, 8
LAST_RESULTS = None


# ---------------------------------------------------------------------------
# workaround: this walrus build rejects >1 sync-wait on a CTRL (Drain) inst.
def _install_tilefix():
    import concourse.tile as tile_mod
    from concourse.vector_clock import ScopedClock, VectorClock

    if getattr(tile_mod.TileContext, "_drain_split_installed", False):
        return

    def _split_drain_and_barrier(self, tick_clock, wait_clock):
        gc = tick_clock.global_clock
        ticks = list(gc)
        nz = [(i, t) for i, t in enumerate(ticks) if t > 0]
        if nz:
            for i, t in nz:
                vec = [0] * len(ticks)
                vec[i] = t
                d = self.nc.sync.drain()
                wait_clock.add_sem_waits(
                    d.ins, ScopedClock({None: VectorClock(vec)})
                )
        else:
            d = self.nc.sync.drain()
            wait_clock.add_sem_waits(d.ins, ScopedClock({None: gc}))
        self.nc.all_engine_barrier()
        assert self.sems is not None
        popped = self.nc._tile_sem_poison_stack.pop()
        assert popped is self._sem_poison
        self.nc.clear_and_free_semaphores(list(self.sems.allocated().values()))
        self.nc.all_engine_barrier()

    tile_mod.TileContext._drain_and_barrier = _split_drain_and_barrier
    tile_mod.TileContext._drain_split_installed = True


# ---------------------------------------------------------------------------
def _wrap_idx(seq):
    """int16 index layout for dma_gather: position i -> [i%16, i//16],
    replicated across the 8 groups of 16 partitions."""
    a = np.asarray(seq, np.int16)
    L = len(a)
    assert L % 16 == 0
    a16 = a.reshape(L // 16, 16).T  # [16, L/16]
    return np.ascontiguousarray(np.tile(a16, (8, 1)))  # [128, L/16]


def _pack_bundles(items, bcap):
    """Pack (key, idx_list) items into bundles of <= bcap indices each.
    Returns (bundles, where): bundles = [concatenated index array ...];
    where[key] = (bundle_idx, offset, length)."""
    bundles = []
    where = {}
    cur, cur_len = [], 0
    for key, lst in items:
        n = len(lst)
        assert n <= bcap
        if cur_len + n > bcap and cur_len > 0:
            bundles.append(np.concatenate(cur))
            cur, cur_len = [], 0
        where[key] = (len(bundles), cur_len, n)
        cur.append(lst)
        cur_len += n
    if cur_len:
        bundles.append(np.concatenate(cur))
    return bundles, where


def _prepare(inputs, D, P, CH, NCH):
    """Host-side index preprocessing. Only index tensors and statically
    known embedding/weight reorders are transformed on the host; all
    batch-dependent float compute stays on device."""
    PP = CH * NCH
    node_indices = np.asarray(inputs["node_indices"])
    parent_indices = np.asarray(inputs["parent_indices"])
    k = (parent_indices > 0).sum(-1)  # [D, P]

    BCAP = 768
    remap = np.zeros(2 + D * P, np.int64)
    remap[1] = 1
    perms = []
    k_olds = []
    for d in range(D):
        if d == 0:
            perm = np.argsort(-k[0], kind="stable")
            k_old = np.zeros(P, np.int64)
        else:
            base_prev_orig = 2 + (d - 1) * P  # first original index of depth d-1
            k_old = ((parent_indices[d] > 0) & (parent_indices[d] < base_prev_orig)).sum(-1)
            perm = np.lexsort((-k[d], -k_old))
        perms.append(perm)
        k_olds.append(k_old)
        remap[2 + d * P + perm] = 2 + d * PP + np.arange(P)

    # Per depth: old bundles (parents in depths <= d-2; per-chunk per-slot
    # prefix lists) and new bundles (parents in depth d-1; per-(slot,block)
    # prefix segments). meta[d] = dict(bundles=[(L, maxref, is_old)...],
    # chunks=[ops...]) where ops are DVE add/copy instructions over pv.
    meta = []
    pidx_wrapped = {}  # (d, bi) -> wrapped idx
    MAXB = 0
    LBMAX = 128
    for d in range(D):
        if d == 0:
            meta.append({"bundles": [], "chunks": [None] * NCH})
            continue
        perm = perms[d]
        kd = k[d][perm]
        kod = k_olds[d][perm]
        knd = kd - kod
        old_items = []  # ((c, 'o', j), list)
        new_items = []  # ((c, 'n', sid), list)
        chunk_ops = []  # per chunk: list of op dicts
        for c in range(NCH):
            lo = c * CH
            hi = min(lo + CH, P)
            nodes = perm[lo:hi]
            ko = kod[lo:hi]
            kn = knd[lo:hi]
            ops = []
            # --- old: slot-major prefix lists
            m_old = [int((ko > j).sum()) for j in range(MP)]
            for j in range(MP):
                if m_old[j] == 0:
                    break
                lst = remap[parent_indices[d, nodes[: m_old[j]], j]]
                old_items.append(((c, "o", j), lst))
            m0, m1 = m_old[0], m_old[1]
            if m1 > 0:
                ops.append({"op": "add2", "lo": 0, "n": m1,
                            "a": (c, "o", 0), "b": (c, "o", 1)})
            if m0 > m1:
                ops.append({"op": "copy", "lo": m1, "n": m0 - m1,
                            "a": (c, "o", 0), "ao": m1})
            if m0 < CH:
                ops.append({"op": "zero", "lo": m0, "n": CH - m0})
            for j in range(2, MP):
                if m_old[j] == 0:
                    break
                ops.append({"op": "add", "lo": 0, "n": m_old[j], "a": (c, "o", j)})
            # --- new: per (new-slot jj, k_old block) prefix segments
            # block boundaries: runs of equal ko (desc); within runs kn desc.
            sid = 0
            bounds = [0] + list(np.nonzero(np.diff(ko))[0] + 1) + [hi - lo]
            for jj in range(MP):
                any_seg = False
                for bs, be in zip(bounds[:-1], bounds[1:]):
                    cnt = int((kn[bs:be] > jj).sum())
                    if cnt == 0:
                        continue
                    any_seg = True
                    K = int(ko[bs])
                    lst = remap[parent_indices[d, nodes[bs : bs + cnt], K + jj]]
                    new_items.append(((c, "n", sid), lst))
                    ops.append({"op": "add", "lo": bs, "n": cnt, "a": (c, "n", sid)})
                    sid += 1
                if not any_seg:
                    break
            chunk_ops.append(ops)

        old_bundles, old_where = _pack_bundles(old_items, BCAP)
        new_bundles, new_where = _pack_bundles(new_items, BCAP)
        nold = len(old_bundles)
        where = {kk: (bi, off, n) for kk, (bi, off, n) in old_where.items()}
        where.update(
            {kk: (nold + bi, off, n) for kk, (bi, off, n) in new_where.items()}
        )
        bundles = []
        for bi, idx in enumerate(old_bundles + new_bundles):
            L = (len(idx) + 127) // 128 * 128
            maxref = int(idx.max())
            idx = np.pad(idx, (0, L - len(idx)))
            pidx_wrapped[(d, bi)] = _wrap_idx(idx)
            bundles.append((L, maxref, bi < nold))
            LBMAX = max(LBMAX, L)
        MAXB = max(MAXB, len(bundles))
        meta.append({"bundles": bundles, "chunks": chunk_ops, "where": where,
                     "nold": nold})

    pidx_np = np.zeros((D, MAXB, 128, LBMAX // 16), np.int16)
    for (d, bi), w in pidx_wrapped.items():
        pidx_np[d, bi, :, : w.shape[1]] = w

    W1 = np.asarray(inputs["W1"], np.float32)
    W2 = np.asarray(inputs["W2"], np.float32)
    Wout = np.asarray(inputs["Wout"], np.float32)
    emb = np.asarray(inputs["emb_table"], np.float32)

    # permuted node-embedding tiles, feature-major [D, NCH, E, CH] (bf16)
    neT = np.zeros((D, NCH, H, CH), BF16)
    for d in range(D):
        for c in range(NCH):
            lo = c * CH
            hi = min(lo + CH, P)
            nodes = perms[d][lo:hi]
            ne = emb[node_indices[d][nodes]]  # [nreal, E] f32
            neT[d, c, :, : hi - lo] = ne.T.astype(BF16)

    wout_perm = np.zeros((D, PP, H), BF16)
    wo = Wout[1:].reshape(D, P, H)
    for d in range(D):
        wout_perm[d, :P] = wo[d][perms[d]].astype(BF16)

    # depth-0 parent counts in permuted order, replicated across partitions
    kcnt = np.zeros((NCH, 128, CH), BF16)
    k0 = np.pad(k[0][perms[0]].astype(np.float32), (0, PP - P))
    for c in range(NCH):
        kcnt[c] = np.tile(k0[c * CH : (c + 1) * CH].astype(BF16), (128, 1))

    prep = {
        "meta": meta,
        "perms": perms,
        "pidx": pidx_np,
        "neT": neT,
        "w1at": np.ascontiguousarray(W1[:, :H].T.astype(BF16)),
        "w1bt": np.ascontiguousarray(W1[:, H:].T.astype(BF16)),
        "w2t": np.ascontiguousarray(W2.T.astype(BF16)),
        "ident": np.eye(128, dtype=BF16),
        "b1": np.asarray(inputs["b1"], np.float32).reshape(128, 1),
        "b2": np.asarray(inputs["b2"], np.float32).reshape(128, 1),
        "woutp": wout_perm,
        "kcnt": kcnt,
    }
    return prep


def _build(prep, D, P, CH, NCH):
    """Trace the Bass/Tile kernel. Returns a finalized Bacc."""
    _install_tilefix()
    from contextlib import ExitStack

    import concourse.bacc as bacc
    import concourse.mybir as mybir
    from concourse.tile import TileContext

    PP = CH * NCH
    TOK = 2 + D * PP
    ROW = BL * H  # nv row elems (bf16)
    KB = CH // 128  # 128-blocks per chunk
    f32 = mybir.dt.float32
    bf16 = mybir.dt.bfloat16
    i16 = mybir.dt.int16
    AF = mybir.ActivationFunctionType
    ALU = mybir.AluOpType
    AX = mybir.AxisListType

    nc = bacc.Bacc("TRN2", target_bir_lowering=False, debug=False)

    nv = nc.dram_tensor("nv", [TOK, ROW], bf16, kind="Internal")
    nvinit = nc.dram_tensor("nvinit", [2, ROW], bf16, kind="ExternalInput")
    rootT_in = nc.dram_tensor("rootT", [128, BL], bf16, kind="ExternalInput")
    neT_in = nc.dram_tensor(
        "neT", list(prep["neT"].shape), bf16, kind="ExternalInput"
    )
    pidx_in = nc.dram_tensor(
        "pidx", list(prep["pidx"].shape), i16, kind="ExternalInput"
    )
    w1at_in = nc.dram_tensor("w1at", [128, 128], bf16, kind="ExternalInput")
    w1bt_in = nc.dram_tensor("w1bt", [128, 128], bf16, kind="ExternalInput")
    w2t_in = nc.dram_tensor("w2t", [128, 128], bf16, kind="ExternalInput")
    ident_in = nc.dram_tensor("ident", [128, 128], bf16, kind="ExternalInput")
    b1_in = nc.dram_tensor("b1c", [128, 1], f32, kind="ExternalInput")
    b2_in = nc.dram_tensor("b2c", [128, 1], f32, kind="ExternalInput")
    woutp_in = nc.dram_tensor("woutp", [D, PP, H], bf16, kind="ExternalInput")
    kcnt_in = nc.dram_tensor("kcnt", [NCH, 128, CH], bf16, kind="ExternalInput")
    outd = nc.dram_tensor("outd", [D, NCH, 128, KB, BL], bf16, kind="ExternalOutput")

    meta = prep["meta"]

    with TileContext(nc) as tc, ExitStack() as ctx:
        const = ctx.enter_context(tc.tile_pool(name="const", bufs=1))
        pidx_pool = ctx.enter_context(tc.tile_pool(name="pidx", bufs=2))
        stag_pool = ctx.enter_context(tc.tile_pool(name="stag", bufs=2))
        pv_pool = ctx.enter_context(tc.tile_pool(name="pv", bufs=2))
        ne_pool = ctx.enter_context(tc.tile_pool(name="ne", bufs=2))
        h1_pool = ctx.enter_context(tc.tile_pool(name="h1", bufs=2))
        nvn_pool = ctx.enter_context(tc.tile_pool(name="nvn", bufs=4))
        nvrm_pool = ctx.enter_context(tc.tile_pool(name="nvrm", bufs=2))
        wout_pool = ctx.enter_context(tc.tile_pool(name="wout", bufs=2))
        outsb_pool = ctx.enter_context(tc.tile_pool(name="outsb", bufs=2))
        tmp_pool = ctx.enter_context(tc.tile_pool(name="tmp", bufs=1))
        psum_mm = ctx.enter_context(tc.tile_pool(name="psmm", bufs=2, space="PSUM"))
        psum_mm2 = ctx.enter_context(tc.tile_pool(name="psm2", bufs=2, space="PSUM"))
        psum_tp = ctx.enter_context(tc.tile_pool(name="pstp", bufs=4, space="PSUM"))

        w1at = const.tile([128, 128], bf16)
        nc.sync.dma_start(out=w1at[:], in_=w1at_in[:, :])
        w1bt = const.tile([128, 128], bf16)
        nc.sync.dma_start(out=w1bt[:], in_=w1bt_in[:, :])
        w2t = const.tile([128, 128], bf16)
        nc.sync.dma_start(out=w2t[:], in_=w2t_in[:, :])
        ident = const.tile([128, 128], bf16)
        nc.sync.dma_start(out=ident[:], in_=ident_in[:, :])
        b1 = const.tile([128, 1], f32)
        nc.sync.dma_start(out=b1[:], in_=b1_in[:, :])
        b2 = const.tile([128, 1], f32)
        nc.sync.dma_start(out=b2[:], in_=b2_in[:, :])
        rootT = const.tile([128, BL], bf16)
        nc.sync.dma_start(out=rootT[:], in_=rootT_in[:, :])

        # init nv rows 0..1 (zero pad row + root = per-b embedding)
        nvi = const.tile([2, ROW], bf16)
        nc.sync.dma_start(out=nvi[:], in_=nvinit[:, :])
        nc.sync.dma_start(out=nv[0:2, :], in_=nvi[:])

        stags = {}  # d -> list of stag tiles
        for d in range(D):
            md = meta[d]
            # ---- gathers for this depth (old bundles first: they only
            # depend on write-backs through depth d-2, so the gather engine
            # runs them during depth d-1's compute).
            dst = []
            for bi, (L, maxref, is_old) in enumerate(md["bundles"]):
                pidx_sb = pidx_pool.tile([128, L // 16], i16, tag=f"pidx{bi}")
                # issue on the gather engine's own queue: on sync it would
                # trail the previous depth's write-backs and delay the train.
                nc.gpsimd.dma_start(out=pidx_sb[:], in_=pidx_in[d, bi, :, : L // 16])
                stag = stag_pool.tile([128, BL, L], bf16, tag=f"gb{bi}")
                nc.gpsimd.dma_gather(
                    stag[:], nv[0 : maxref + 1, :], pidx_sb[:],
                    num_idxs=L, num_idxs_reg=L,
                    elem_size=ROW, transpose=True,
                )
                dst.append(stag)
            stags[d] = dst

            for c in range(NCH):
                # ---- parent-sum -> pv bf16 [128, BL, CH]
                pv = pv_pool.tile([128, BL, CH], bf16)
                if d == 0:
                    kc = ne_pool.tile([128, 1, CH], bf16, tag="kcnt")
                    nc.sync.dma_start(out=kc[:], in_=kcnt_in[c, :, :].unsqueeze(1))
                    nc.vector.tensor_tensor(
                        out=pv[:],
                        in0=rootT[:].unsqueeze(2).to_broadcast([128, BL, CH]),
                        in1=kc[:].to_broadcast([128, BL, CH]),
                        op=ALU.mult,
                    )
                else:
                    where = md["where"]

                    def seg(key, off0, n):
                        bi, off, ln = where[key]
                        assert off0 + n <= ln
                        return stags[d][bi][:, :, off + off0 : off + off0 + n]

                    for op in md["chunks"][c]:
                        lo, n = op["lo"], op["n"]
                        dstv = pv[:, :, lo : lo + n]
                        if op["op"] == "add2":
                            nc.vector.tensor_add(
                                dstv, seg(op["a"], 0, n), seg(op["b"], 0, n)
                            )
                        elif op["op"] == "copy":
                            nc.vector.tensor_copy(
                                out=dstv, in_=seg(op["a"], op["ao"], n)
                            )
                        elif op["op"] == "zero":
                            nc.vector.memset(dstv, 0)
                        elif op["op"] == "add":
                            nc.vector.tensor_add(dstv, dstv, seg(op["a"], 0, n))

                ne = ne_pool.tile([128, 1, CH], bf16)
                nc.sync.dma_start(out=ne[:], in_=neT_in[d, c, :, :].unsqueeze(1))

                # ---- MLP (bf16) over col pairs (2 b's x CH = 512 cols)
                nvns = []
                for bp in range(BL // 2):
                    rhs_pv = pv[:, 2 * bp : 2 * bp + 2, :]
                    h1p = psum_mm.tile([128, 2, CH], f32, tag="h1p")
                    nc.tensor.matmul(
                        h1p[:], lhsT=w1at[:], rhs=rhs_pv, start=True, stop=False
                    )
                    nc.tensor.matmul(
                        h1p[:],
                        lhsT=w1bt[:],
                        rhs=ne[:].to_broadcast([128, 2, CH]),
                        start=False,
                        stop=True,
                    )
                    h1 = h1_pool.tile([128, 2, CH], bf16)
                    nc.scalar.activation(h1[:], h1p[:], AF.Relu, bias=b1[:])
                    h2p = psum_mm2.tile([128, 2, CH], f32, tag="h2p")
                    nc.tensor.matmul(
                        h2p[:], lhsT=w2t[:], rhs=h1[:], start=True, stop=False
                    )
                    nc.tensor.matmul(
                        h2p[:], lhsT=ident[:], rhs=rhs_pv, start=False, stop=True
                    )
                    nvn = nvn_pool.tile([128, 2, CH], bf16)
                    nc.scalar.activation(nvn[:], h2p[:], AF.Identity, bias=b2[:])
                    nvns.append(nvn)

                # ---- transpose back (bf16), write-back, fused out-projection
                outsb = outsb_pool.tile([128, KB * BL], bf16)
                for kb in range(KB):
                    kbg = c * KB + kb
                    wout_sb = wout_pool.tile([128, 128], bf16)
                    nc.sync.dma_start(
                        out=wout_sb[:],
                        in_=woutp_in[d, kbg * 128 : (kbg + 1) * 128, :],
                    )
                    nvrm = nvrm_pool.tile([128, BL, 128], bf16)
                    for half in range(2):
                        tp = psum_tp.tile([128, 4, 128], bf16, tag="tp")
                        for bq in range(4):
                            b = half * 4 + bq
                            nc.tensor.transpose(
                                tp[:, bq, :],
                                nvns[b // 2][:, b % 2, kb * 128 : (kb + 1) * 128],
                                ident[:],
                            )
                        nc.scalar.copy(
                            out=nvrm[:, half * 4 : half * 4 + 4, :], in_=tp[:]
                        )
                    # out-projection: out[t, b] = sum_h nvrm[t,b,h]*wout[t,h]
                    tmp = tmp_pool.tile([128, BL, 128], bf16)
                    nc.vector.tensor_tensor(
                        out=tmp[:],
                        in0=nvrm[:],
                        in1=wout_sb[:].unsqueeze(1).to_broadcast([128, BL, 128]),
                        op=ALU.mult,
                    )
                    with nc.allow_low_precision("bf16 out-projection partial"):
                        nc.vector.tensor_reduce(
                            out=outsb[:, kb * BL : (kb + 1) * BL],
                            in_=tmp[:],
                            axis=AX.X,
                            op=ALU.add,
                        )
                    tokbase = 2 + d * PP + c * CH + kb * 128
                    nc.sync.dma_start(
                        out=nv[tokbase : tokbase + 128, :],
                        in_=nvrm[:].rearrange("p b h -> p (b h)"),
                    )
                nc.sync.dma_start(
                    out=outd[d, c, :, :, :],
                    in_=outsb[:].rearrange("p (k b) -> p k b", k=KB),
                )

    nc.finalize()
    return nc


def _run_cores(nc, prep, embedding, n_cores):
    from concourse import bass_utils

    in_maps = []
    base = {
        "neT": prep["neT"],
        "pidx": prep["pidx"],
        "w1at": prep["w1at"],
        "w1bt": prep["w1bt"],
        "w2t": prep["w2t"],
        "ident": prep["ident"],
        "b1c": prep["b1"],
        "b2c": prep["b2"],
        "woutp": prep["woutp"],
        "kcnt": prep["kcnt"],
    }
    for core in range(n_cores):
        eb = embedding[core * BL : (core + 1) * BL]  # [BL, H]
        nvinit = np.zeros((2, BL * H), np.float32)
        nvinit[1] = eb.reshape(-1)
        m = dict(base)
        m["nvinit"] = np.ascontiguousarray(nvinit.astype(BF16))
        m["rootT"] = np.ascontiguousarray(eb.T.astype(BF16))
        in_maps.append(m)
    res = bass_utils.run_bass_kernel_spmd(
        nc, in_maps, core_ids=list(range(n_cores))
    )
    global LAST_RESULTS
    LAST_RESULTS = res
    return res


def _assemble(results, prep, inputs, D, P, CH, NCH, n_cores):
    PP = CH * NCH
    KB = CH // 128
    embedding = np.asarray(inputs["embedding"], np.float32)
    Wout = np.asarray(inputs["Wout"], np.float32)
    bout = np.asarray(inputs["bout"], np.float32)
    NTOT = 1 + D * P

    out = np.empty((embedding.shape[0], NTOT), np.float32)
    out[:, 0] = embedding @ Wout[0] + bout[0]
    for core in range(n_cores):
        v = np.asarray(results[core]["outd"], np.float32)  # [D, NCH, 128, KB, BL]
        v = v.transpose(0, 1, 3, 2, 4).reshape(D, PP, BL)  # s = c*CH + kb*128 + n
        for d in range(D):
            perm = prep["perms"][d]
            cols = 1 + d * P + perm  # output column for sorted position s
            out[core * BL : (core + 1) * BL, cols] = v[d, :P].T
    out[:, 1:] += bout[None, 1:]
    return out


def kernel(**inputs):
    D, P, CH, NCH = D_FULL, P_FULL, 256, 4
    prep = _prepare(inputs, D, P, CH, NCH)
    nc = _build(prep, D, P, CH, NCH)
    res = _run_cores(nc, prep, np.asarray(inputs["embedding"], np.float32), NCORES)
    return _assemble(res.results, prep, inputs, D, P, CH, NCH, NCORES)


# revision 21
# speedup vs baseline: 1.1413x; 1.0555x over previous
"""Trainium2 Bass kernel for nn_DAGModel (gnn_message_passing).

Strategy (data-parallel over batch, 8 b's per core):
- node_vecs live in DRAM as a bf16 table `nv[token, b8, h128]` (2KB rows,
  all 8 local batch elements interleaved per token so one gather serves
  all of them).
- Parent gathers use the GPSIMD bulk `dma_gather(transpose=True)` which
  lands feature-major ([h on partitions, gather-position on free]) —
  exactly the matmul layout.
- Each depth's tokens are reordered (host-side renumbering) by
  (#parents-from-older-depths desc, #parents desc). Parents from depths
  <= d-2 ("old") then form per-slot prefix lists, and parents from depth
  d-1 ("new") form per-(slot,block) prefix segments. Old and new refs are
  packed into separate gather bundles: old bundles depend only on
  write-backs through depth d-2, so the gather engine fetches them
  DURING depth d-1's compute; only the small new bundles wait for depth
  d-1's write-back. This removes most of the per-depth serialization.
- Bundles are packed across chunks per depth (fewer gather instructions,
  each <= BCAP indices; the HW path crashes above ~900).
- Depth 0 has only the root as a possible parent: pv = count * root,
  computed on-chip with no gathers.
- Node embeddings are statically known (node_indices is an input), so
  permuted embedding tiles are prepared host-side feature-major and
  streamed in — no embedding gathers.
- The 2-layer MLP runs in bf16 on the PE (h on partitions); the residual
  `pv` is injected into the second matmul's PSUM via an identity matmul.
- New vecs are PE-transposed (bf16, single pass) back to row-major and
  DMA'd to the next depth's token rows; the output projection
  out[t] = nv[t]·Wout[t] is a DVE multiply + reduce over the row-major
  tile, so the final [B,N] readout costs no extra pass over node_vecs.
"""

import numpy as np
import ml_dtypes

BF16 = ml_dtypes.bfloat16

# Full-problem dims (hardcoded per contract).
B, H, E = 64, 128, 128
D_FULL, P_FULL, MP = 20, 1000, 8
NCORES, BL = 8, 8
LAST_RESULTS = None


# ---------------------------------------------------------------------------
# workaround: this walrus build rejects >1 sync-wait on a CTRL (Drain) inst.
def _install_tilefix():
    import concourse.tile as tile_mod
    from concourse.vector_clock import ScopedClock, VectorClock

    if getattr(tile_mod.TileContext, "_drain_split_installed", False):
        return

    def _split_drain_and_barrier(self, tick_clock, wait_clock):
        gc = tick_clock.global_clock
        ticks = list(gc)
        nz = [(i, t) for i, t in enumerate(ticks) if t > 0]
        if nz:
            for i, t in nz:
                vec = [0] * len(ticks)
                vec[i] = t
                d = self.nc.sync.drain()
                wait_clock.add_sem_waits(
                    d.ins, ScopedClock({None: VectorClock(vec)})
                )
        else:
            d = self.nc.sync.drain()
            wait_clock.add_sem_waits(d.ins, ScopedClock({None: gc}))
        self.nc.all_engine_barrier()
        assert self.sems is not None
        popped = self.nc._tile_sem_poison_stack.pop()
        assert popped is self._sem_poison
        self.nc.clear_and_free_semaphores(list(self.sems.allocated().values()))
        self.nc.all_engine_barrier()

    tile_mod.TileContext._drain_and_barrier = _split_drain_and_barrier
    tile_mod.TileContext._drain_split_installed = True


# ---------------------------------------------------------------------------
def _wrap_idx(seq):
    """int16 index layout for dma_gather: position i -> [i%16, i//16],
    replicated across the 8 groups of 16 partitions."""
    a = np.asarray(seq, np.int16)
    L = len(a)
    assert L % 16 == 0
    a16 = a.reshape(L // 16, 16).T  # [16, L/16]
    return np.ascontiguousarray(np.tile(a16, (8, 1)))  # [128, L/16]


def _pack_bundles(items, bcap):
    """Pack (key, idx_list) items into bundles of <= bcap indices each.
    Returns (bundles, where): bundles = [concatenated index array ...];
    where[key] = (bundle_idx, offset, length)."""
    bundles = []
    where = {}
    cur, cur_len = [], 0
    for key, lst in items:
        n = len(lst)
        assert n <= bcap
        if cur_len + n > bcap and cur_len > 0:
            bundles.append(np.concatenate(cur))
            cur, cur_len = [], 0
        where[key] = (len(bundles), cur_len, n)
        cur.append(lst)
        cur_len += n
    if cur_len:
        bundles.append(np.concatenate(cur))
    return bundles, where


def _prepare(inputs, D, P, CH, NCH):
    """Host-side index preprocessing. Only index tensors and statically
    known embedding/weight reorders are transformed on the host; all
    batch-dependent float compute stays on device."""
    PP = CH * NCH
    node_indices = np.asarray(inputs["node_indices"])
    parent_indices = np.asarray(inputs["parent_indices"])
    k = (parent_indices > 0).sum(-1)  # [D, P]

    BCAP = 768
    remap = np.zeros(2 + D * P, np.int64)
    remap[1] = 1
    perms = []
    k_olds = []
    for d in range(D):
        if d == 0:
            perm = np.argsort(-k[0], kind="stable")
            k_old = np.zeros(P, np.int64)
        else:
            base_prev_orig = 2 + (d - 1) * P  # first original index of depth d-1
            k_old = ((parent_indices[d] > 0) & (parent_indices[d] < base_prev_orig)).sum(-1)
            perm = np.lexsort((-k[d], -k_old))
        perms.append(perm)
        k_olds.append(k_old)
        remap[2 + d * P + perm] = 2 + d * PP + np.arange(P)

    # Per depth: old bundles (parents in depths <= d-2; per-chunk per-slot
    # prefix lists) and new bundles (parents in depth d-1; per-(slot,block)
    # prefix segments). meta[d] = dict(bundles=[(L, maxref, is_old)...],
    # chunks=[ops...]) where ops are DVE add/copy instructions over pv.
    meta = []
    pidx_wrapped = {}  # (d, bi) -> wrapped idx
    MAXB = 0
    LBMAX = 128
    for d in range(D):
        if d == 0:
            meta.append({"bundles": [], "chunks": [None] * NCH})
            continue
        perm = perms[d]
        kd = k[d][perm]
        kod = k_olds[d][perm]
        knd = kd - kod
        old_items = []  # ((c, 'o', j), list)
        new_items = []  # ((c, 'n', sid), list)
        chunk_ops = []  # per chunk: list of op dicts
        for c in range(NCH):
            lo = c * CH
            hi = min(lo + CH, P)
            nodes = perm[lo:hi]
            ko = kod[lo:hi]
            kn = knd[lo:hi]
            ops = []
            # --- old: slot-major prefix lists
            m_old = [int((ko > j).sum()) for j in range(MP)]
            for j in range(MP):
                if m_old[j] == 0:
                    break
                lst = remap[parent_indices[d, nodes[: m_old[j]], j]]
                old_items.append(((c, "o", j), lst))
            m0, m1 = m_old[0], m_old[1]
            if m1 > 0:
                ops.append({"op": "add2", "lo": 0, "n": m1,
                            "a": (c, "o", 0), "b": (c, "o", 1)})
            if m0 > m1:
                ops.append({"op": "copy", "lo": m1, "n": m0 - m1,
                            "a": (c, "o", 0), "ao": m1})
            if m0 < CH:
                ops.append({"op": "zero", "lo": m0, "n": CH - m0})
            for j in range(2, MP):
                if m_old[j] == 0:
                    break
                ops.append({"op": "add", "lo": 0, "n": m_old[j], "a": (c, "o", j)})
            # --- new: per (new-slot jj, k_old block) prefix segments
            # block boundaries: runs of equal ko (desc); within runs kn desc.
            sid = 0
            bounds = [0] + list(np.nonzero(np.diff(ko))[0] + 1) + [hi - lo]
            for jj in range(MP):
                any_seg = False
                for bs, be in zip(bounds[:-1], bounds[1:]):
                    cnt = int((kn[bs:be] > jj).sum())
                    if cnt == 0:
                        continue
                    any_seg = True
                    K = int(ko[bs])
                    lst = remap[parent_indices[d, nodes[bs : bs + cnt], K + jj]]
                    new_items.append(((c, "n", sid), lst))
                    ops.append({"op": "add", "lo": bs, "n": cnt, "a": (c, "n", sid)})
                    sid += 1
                if not any_seg:
                    break
            chunk_ops.append(ops)

        old_bundles, old_where = _pack_bundles(old_items, BCAP)
        new_bundles, new_where = _pack_bundles(new_items, BCAP)
        nold = len(old_bundles)
        where = {kk: (bi, off, n) for kk, (bi, off, n) in old_where.items()}
        where.update(
            {kk: (nold + bi, off, n) for kk, (bi, off, n) in new_where.items()}
        )
        bundles = []
        for bi, idx in enumerate(old_bundles + new_bundles):
            L = (len(idx) + 127) // 128 * 128
            maxref = int(idx.max())
            idx = np.pad(idx, (0, L - len(idx)))
            pidx_wrapped[(d, bi)] = _wrap_idx(idx)
            bundles.append((L, maxref, bi < nold))
            LBMAX = max(LBMAX, L)
        MAXB = max(MAXB, len(bundles))
        meta.append({"bundles": bundles, "chunks": chunk_ops, "where": where,
                     "nold": nold})

    pidx_np = np.zeros((D, MAXB, 128, LBMAX // 16), np.int16)
    for (d, bi), w in pidx_wrapped.items():
        pidx_np[d, bi, :, : w.shape[1]] = w

    W1 = np.asarray(inputs["W1"], np.float32)
    W2 = np.asarray(inputs["W2"], np.float32)
    Wout = np.asarray(inputs["Wout"], np.float32)
    emb = np.asarray(inputs["emb_table"], np.float32)

    # permuted node-embedding tiles, feature-major [D, NCH, E, CH] (bf16)
    neT = np.zeros((D, NCH, H, CH), BF16)
    for d in range(D):
        for c in range(NCH):
            lo = c * CH
            hi = min(lo + CH, P)
            nodes = perms[d][lo:hi]
            ne = emb[node_indices[d][nodes]]  # [nreal, E] f32
            neT[d, c, :, : hi - lo] = ne.T.astype(BF16)

    wout_perm = np.zeros((D, PP, H), BF16)
    wo = Wout[1:].reshape(D, P, H)
    for d in range(D):
        wout_perm[d, :P] = wo[d][perms[d]].astype(BF16)

    # depth-0 parent counts in permuted order, replicated across partitions
    kcnt = np.zeros((NCH, 128, CH), BF16)
    k0 = np.pad(k[0][perms[0]].astype(np.float32), (0, PP - P))
    for c in range(NCH):
        kcnt[c] = np.tile(k0[c * CH : (c + 1) * CH].astype(BF16), (128, 1))

    prep = {
        "meta": meta,
        "perms": perms,
        "pidx": pidx_np,
        "neT": neT,
        "w1at": np.ascontiguousarray(W1[:, :H].T.astype(BF16)),
        "w1bt": np.ascontiguousarray(W1[:, H:].T.astype(BF16)),
        "w2t": np.ascontiguousarray(W2.T.astype(BF16)),
        "ident": np.eye(128, dtype=BF16),
        "b1": np.asarray(inputs["b1"], np.float32).reshape(128, 1),
        "b2": np.asarray(inputs["b2"], np.float32).reshape(128, 1),
        "woutp": wout_perm,
        "kcnt": kcnt,
    }
    return prep


def _build(prep, D, P, CH, NCH):
    """Trace the Bass/Tile kernel. Returns a finalized Bacc."""
    _install_tilefix()
    from contextlib import ExitStack

    import concourse.bacc as bacc
    import concourse.mybir as mybir
    from concourse.tile import TileContext

    PP = CH * NCH
    TOK = 2 + D * PP
    ROW = BL * H  # nv row elems (bf16)
    KB = CH // 128  # 128-blocks per chunk
    f32 = mybir.dt.float32
    bf16 = mybir.dt.bfloat16
    i16 = mybir.dt.int16
    AF = mybir.ActivationFunctionType
    ALU = mybir.AluOpType
    AX = mybir.AxisListType

    nc = bacc.Bacc("TRN2", target_bir_lowering=False, debug=False)

    nv = nc.dram_tensor("nv", [TOK, ROW], bf16, kind="Internal")
    nvinit = nc.dram_tensor("nvinit", [2, ROW], bf16, kind="ExternalInput")
    rootT_in = nc.dram_tensor("rootT", [128, BL], bf16, kind="ExternalInput")
    neT_in = nc.dram_tensor(
        "neT", list(prep["neT"].shape), bf16, kind="ExternalInput"
    )
    pidx_in = nc.dram_tensor(
        "pidx", list(prep["pidx"].shape), i16, kind="ExternalInput"
    )
    w1at_in = nc.dram_tensor("w1at", [128, 128], bf16, kind="ExternalInput")
    w1bt_in = nc.dram_tensor("w1bt", [128, 128], bf16, kind="ExternalInput")
    w2t_in = nc.dram_tensor("w2t", [128, 128], bf16, kind="ExternalInput")
    ident_in = nc.dram_tensor("ident", [128, 128], bf16, kind="ExternalInput")
    b1_in = nc.dram_tensor("b1c", [128, 1], f32, kind="ExternalInput")
    b2_in = nc.dram_tensor("b2c", [128, 1], f32, kind="ExternalInput")
    woutp_in = nc.dram_tensor("woutp", [D, PP, H], bf16, kind="ExternalInput")
    kcnt_in = nc.dram_tensor("kcnt", [NCH, 128, CH], bf16, kind="ExternalInput")
    outd = nc.dram_tensor("outd", [D, NCH, 128, KB, BL], bf16, kind="ExternalOutput")

    meta = prep["meta"]

    with TileContext(nc) as tc, ExitStack() as ctx:
        const = ctx.enter_context(tc.tile_pool(name="const", bufs=1))
        pidx_pool = ctx.enter_context(tc.tile_pool(name="pidx", bufs=2))
        stag_pool = ctx.enter_context(tc.tile_pool(name="stag", bufs=2))
        pv_pool = ctx.enter_context(tc.tile_pool(name="pv", bufs=2))
        ne_pool = ctx.enter_context(tc.tile_pool(name="ne", bufs=2))
        h1_pool = ctx.enter_context(tc.tile_pool(name="h1", bufs=2))
        nvn_pool = ctx.enter_context(tc.tile_pool(name="nvn", bufs=4))
        nvrm_pool = ctx.enter_context(tc.tile_pool(name="nvrm", bufs=2))
        wout_pool = ctx.enter_context(tc.tile_pool(name="wout", bufs=2))
        outsb_pool = ctx.enter_context(tc.tile_pool(name="outsb", bufs=2))
        tmp_pool = ctx.enter_context(tc.tile_pool(name="tmp", bufs=1))
        psum_mm = ctx.enter_context(tc.tile_pool(name="psmm", bufs=2, space="PSUM"))
        psum_mm2 = ctx.enter_context(tc.tile_pool(name="psm2", bufs=2, space="PSUM"))
        psum_tp = ctx.enter_context(tc.tile_pool(name="pstp", bufs=4, space="PSUM"))

        w1at = const.tile([128, 128], bf16)
        nc.sync.dma_start(out=w1at[:], in_=w1at_in[:, :])
        w1bt = const.tile([128, 128], bf16)
        nc.sync.dma_start(out=w1bt[:], in_=w1bt_in[:, :])
        w2t = const.tile([128, 128], bf16)
        nc.sync.dma_start(out=w2t[:], in_=w2t_in[:, :])
        ident = const.tile([128, 128], bf16)
        nc.sync.dma_start(out=ident[:], in_=ident_in[:, :])
        b1 = const.tile([128, 1], f32)
        nc.sync.dma_start(out=b1[:], in_=b1_in[:, :])
        b2 = const.tile([128, 1], f32)
        nc.sync.dma_start(out=b2[:], in_=b2_in[:, :])
        rootT = const.tile([128, BL], bf16)
        nc.sync.dma_start(out=rootT[:], in_=rootT_in[:, :])

        # init nv rows 0..1 (zero pad row + root = per-b embedding)
        nvi = const.tile([2, ROW], bf16)
        nc.sync.dma_start(out=nvi[:], in_=nvinit[:, :])
        nc.sync.dma_start(out=nv[0:2, :], in_=nvi[:])

        MAXB = prep["pidx"].shape[1]
        W = prep["pidx"].shape[3]

        def load_pidx(dd):
            """One DMA for all of depth dd's bundle indices, issued on the
            sync queue a depth ahead so it never trails the write-backs."""
            t = pidx_pool.tile([128, MAXB, W], i16, tag="pidx")
            nc.sync.dma_start(
                out=t[:], in_=pidx_in[dd].rearrange("m p w -> p m w")
            )
            return t

        pidx_tiles = {}
        stags = {}  # d -> list of stag tiles
        for d in range(D):
            md = meta[d]
            if d + 1 < D:
                pidx_tiles[d + 1] = load_pidx(d + 1)
            # ---- gathers for this depth (old bundles first: they only
            # depend on write-backs through depth d-2, so the gather engine
            # runs them during depth d-1's compute).
            dst = []
            for bi, (L, maxref, is_old) in enumerate(md["bundles"]):
                pidx_sb = pidx_tiles[d]
                stag = stag_pool.tile([128, BL, L], bf16, tag=f"gb{bi}")
                nc.gpsimd.dma_gather(
                    stag[:], nv[0 : maxref + 1, :], pidx_sb[:, bi, : L // 16],
                    num_idxs=L, num_idxs_reg=L,
                    elem_size=ROW, transpose=True,
                )
                dst.append(stag)
            stags[d] = dst

            for c in range(NCH):
                # ---- parent-sum -> pv bf16 [128, BL, CH]
                pv = pv_pool.tile([128, BL, CH], bf16)
                if d == 0:
                    kc = ne_pool.tile([128, 1, CH], bf16, tag="kcnt")
                    nc.sync.dma_start(out=kc[:], in_=kcnt_in[c, :, :].unsqueeze(1))
                    nc.vector.tensor_tensor(
                        out=pv[:],
                        in0=rootT[:].unsqueeze(2).to_broadcast([128, BL, CH]),
                        in1=kc[:].to_broadcast([128, BL, CH]),
                        op=ALU.mult,
                    )
                else:
                    where = md["where"]

                    def seg(key, off0, n):
                        bi, off, ln = where[key]
                        assert off0 + n <= ln
                        return stags[d][bi][:, :, off + off0 : off + off0 + n]

                    for op in md["chunks"][c]:
                        lo, n = op["lo"], op["n"]
                        dstv = pv[:, :, lo : lo + n]
                        if op["op"] == "add2":
                            nc.vector.tensor_add(
                                dstv, seg(op["a"], 0, n), seg(op["b"], 0, n)
                            )
                        elif op["op"] == "copy":
                            nc.vector.tensor_copy(
                                out=dstv, in_=seg(op["a"], op["ao"], n)
                            )
                        elif op["op"] == "zero":
                            nc.vector.memset(dstv, 0)
                        elif op["op"] == "add":
                            nc.vector.tensor_add(dstv, dstv, seg(op["a"], 0, n))

                ne = ne_pool.tile([128, 1, CH], bf16)
                nc.sync.dma_start(out=ne[:], in_=neT_in[d, c, :, :].unsqueeze(1))

                # ---- MLP (bf16) over col pairs (2 b's x CH = 512 cols)
                nvns = []
                for bp in range(BL // 2):
                    rhs_pv = pv[:, 2 * bp : 2 * bp + 2, :]
                    h1p = psum_mm.tile([128, 2, CH], f32, tag="h1p")
                    nc.tensor.matmul(
                        h1p[:], lhsT=w1at[:], rhs=rhs_pv, start=True, stop=False
                    )
                    nc.tensor.matmul(
                        h1p[:],
                        lhsT=w1bt[:],
                        rhs=ne[:].to_broadcast([128, 2, CH]),
                        start=False,
                        stop=True,
                    )
                    h1 = h1_pool.tile([128, 2, CH], bf16)
                    nc.scalar.activation(h1[:], h1p[:], AF.Relu, bias=b1[:])
                    h2p = psum_mm2.tile([128, 2, CH], f32, tag="h2p")
                    nc.tensor.matmul(
                        h2p[:], lhsT=w2t[:], rhs=h1[:], start=True, stop=False
                    )
                    nc.tensor.matmul(
                        h2p[:], lhsT=ident[:], rhs=rhs_pv, start=False, stop=True
                    )
                    nvn = nvn_pool.tile([128, 2, CH], bf16)
                    nc.scalar.activation(nvn[:], h2p[:], AF.Identity, bias=b2[:])
                    nvns.append(nvn)

                # ---- transpose back (bf16), write-back, fused out-projection
                outsb = outsb_pool.tile([128, KB * BL], bf16)
                for kb in range(KB):
                    kbg = c * KB + kb
                    wout_sb = wout_pool.tile([128, 128], bf16)
                    nc.sync.dma_start(
                        out=wout_sb[:],
                        in_=woutp_in[d, kbg * 128 : (kbg + 1) * 128, :],
                    )
                    nvrm = nvrm_pool.tile([128, BL, 128], bf16)
                    for half in range(2):
                        tp = psum_tp.tile([128, 4, 128], bf16, tag="tp")
                        for bq in range(4):
                            b = half * 4 + bq
                            nc.tensor.transpose(
                                tp[:, bq, :],
                                nvns[b // 2][:, b % 2, kb * 128 : (kb + 1) * 128],
                                ident[:],
                            )
                        nc.scalar.copy(
                            out=nvrm[:, half * 4 : half * 4 + 4, :], in_=tp[:]
                        )
                    # out-projection: out[t, b] = sum_h nvrm[t,b,h]*wout[t,h]
                    tmp = tmp_pool.tile([128, BL, 128], bf16)
                    nc.vector.tensor_tensor(
                        out=tmp[:],
                        in0=nvrm[:],
                        in1=wout_sb[:].unsqueeze(1).to_broadcast([128, BL, 128]),
                        op=ALU.mult,
                    )
                    with nc.allow_low_precision("bf16 out-projection partial"):
                        nc.vector.tensor_reduce(
                            out=outsb[:, kb * BL : (kb + 1) * BL],
                            in_=tmp[:],
                            axis=AX.X,
                            op=ALU.add,
                        )
                    tokbase = 2 + d * PP + c * CH + kb * 128
                    nc.sync.dma_start(
                        out=nv[tokbase : tokbase + 128, :],
                        in_=nvrm[:].rearrange("p b h -> p (b h)"),
                    )
                nc.sync.dma_start(
                    out=outd[d, c, :, :, :],
                    in_=outsb[:].rearrange("p (k b) -> p k b", k=KB),
                )

    nc.finalize()
    return nc


def _run_cores(nc, prep, embedding, n_cores):
    from concourse import bass_utils

    in_maps = []
    base = {
        "neT": prep["neT"],
        "pidx": prep["pidx"],
        "w1at": prep["w1at"],
        "w1bt": prep["w1bt"],
        "w2t": prep["w2t"],
        "ident": prep["ident"],
        "b1c": prep["b1"],
        "b2c": prep["b2"],
        "woutp": prep["woutp"],
        "kcnt": prep["kcnt"],
    }
    for core in range(n_cores):
        eb = embedding[core * BL : (core + 1) * BL]  # [BL, H]
        nvinit = np.zeros((2, BL * H), np.float32)
        nvinit[1] = eb.reshape(-1)
        m = dict(base)
        m["nvinit"] = np.ascontiguousarray(nvinit.astype(BF16))
        m["rootT"] = np.ascontiguousarray(eb.T.astype(BF16))
        in_maps.append(m)
    res = bass_utils.run_bass_kernel_spmd(
        nc, in_maps, core_ids=list(range(n_cores))
    )
    global LAST_RESULTS
    LAST_RESULTS = res
    return res


def _assemble(results, prep, inputs, D, P, CH, NCH, n_cores):
    PP = CH * NCH
    KB = CH // 128
    embedding = np.asarray(inputs["embedding"], np.float32)
    Wout = np.asarray(inputs["Wout"], np.float32)
    bout = np.asarray(inputs["bout"], np.float32)
    NTOT = 1 + D * P

    out = np.empty((embedding.shape[0], NTOT), np.float32)
    out[:, 0] = embedding @ Wout[0] + bout[0]
    for core in range(n_cores):
        v = np.asarray(results[core]["outd"], np.float32)  # [D, NCH, 128, KB, BL]
        v = v.transpose(0, 1, 3, 2, 4).reshape(D, PP, BL)  # s = c*CH + kb*128 + n
        for d in range(D):
            perm = prep["perms"][d]
            cols = 1 + d * P + perm  # output column for sorted position s
            out[core * BL : (core + 1) * BL, cols] = v[d, :P].T
    out[:, 1:] += bout[None, 1:]
    return out


def kernel(**inputs):
    D, P, CH, NCH = D_FULL, P_FULL, 256, 4
    prep = _prepare(inputs, D, P, CH, NCH)
    nc = _build(prep, D, P, CH, NCH)
    res = _run_cores(nc, prep, np.asarray(inputs["embedding"], np.float32), NCORES)
    return _assemble(res.results, prep, inputs, D, P, CH, NCH, NCORES)
